# revision 19
# baseline (speedup 1.0000x reference)
"""Mistral attention TP-8 Bass kernel builder.

Per-core program (core c of 8):
  inputs (bf16 unless noted):
    xt   [H, S]     hidden_states^T
    wqkv [H, 768]   [Wq_c | Wk_c | Wv_c] columns (4 q heads + 1 kv head)
    wo   [512, H]   Wo rows for this core's heads
    cq,sq,ck,sk [128, S]  rope tables^T (bf16), q-tables pre-scaled by 1/sqrt(128),
                    s-tables sign-baked (rows 0:64 negative)
    cmask [128, 4, 512] (causal mode only) diagonal 0/1 patterns
    maskT [S, S] fp32 (general mode only) additive mask transposed
  output:
    outT [H, S] fp32   partial out^T (host sums over cores, transposes)

Compute (all matmuls bf16, fp32 PSUM accumulation):
  qkv^T = wqkv^T-tiles @ xt-tiles      -> [768, S] in [d-part, s-free] layout
  rope on q (scaled 1/sqrt(d)), k
  v transposed via PE into Vtil [s_k-part, 16, 132] with ones col 128
  scores^T tile = kT-tile (lhsT) x qT  -> psum [s_k 128, s_q 512]; ACT exp -> bf16
  PV: lhsT = expP tile [s_k, s_q 128], rhs = Vtil [s_k, 129] accum over s_k
      -> psum [s_q 128, 129], col 128 = softmax denominator
  normalize rows by 1/denom (per-partition), transpose -> attnT [d, s]
  o_proj: lhsT = wo tiles, rhs = attnT -> outT [H, S] fp32
"""
import sys
sys.path.insert(0, '/opt/trn_rl_repo')

import math
import numpy as np
import ml_dtypes

import concourse.bass as bass
import concourse.mybir as mybir
import concourse.tile as tile
from concourse import bacc
from concourse.masks import make_identity

FP32 = mybir.dt.float32
BF16 = mybir.dt.bfloat16
AF = mybir.ActivationFunctionType
OP = mybir.AluOpType

H = 4096
HD = 128
NH = 32
NKV = 8
NCORES = 8
HQ = NH // NCORES          # 4 q heads per core
NQKV = HQ + 2              # 6 projection n-tiles (4 q, 1 k, 1 v)
QKV_N = NQKV * HD          # 768


def build_nc(S=2048, mode="none", loop_k=0, internal_io=False, phases=3,
             opts=None):
    """mode: 'none' (no mask), 'causal', 'general'. loop_k>0 wraps body in For_i.

    internal_io=True makes all big tensors Internal DRAM scratch (garbage
    values, zero host transfer) for pure device timing runs; only a tiny
    dummy in/out pair remains external.
    """
    # defaults = best measured configuration, so that timing harnesses that
    # call build_nc() without opts measure the optimized kernel.
    o = dict(psA_bufs=8, pstA_bufs=2, xpool_bufs=2, rope="full", po_bufs=2,
             pv_bufs=2, sc2_bufs=3, pst2_bufs=1, expp_bufs=2,
             sc2_dtype="fp32", split_tr=False, offset_scores=True,
             evac_pool=False, ch=512, tr_dma=True, vtr_dma=False,
             wo_pair=False, proj="nchain", pj_bufs=3, xg=4,
             pv="flip", pvf_bufs=1, den_bufs=1,
             no_oproj=False, no_pv=False, exp_dve=False,
             out_bf16=True, evac_any=False, evac_tiny=False, wo_res=True)
    o.update(opts or {})
    assert S % 512 == 0
    NSC = S // 512            # s chunks (projection phase)
    NKT = S // 128            # s_k tiles
    HT = H // 128             # h tiles (contraction for qkv proj)
    MT = H // 128             # output m tiles
    CH = o["ch"]              # attention s_q chunk
    NCH = S // CH
    QTN = CH // 128
    SCN = CH // 512

    nc = bacc.Bacc("TRN2", target_bir_lowering=False, debug=False,
                   num_devices=NCORES)

    ikind = dict() if internal_io else dict(kind="ExternalInput")
    okind = dict() if internal_io else dict(kind="ExternalOutput")
    xt = nc.dram_tensor("xt", [H, S], BF16, **ikind)
    wqkv = nc.dram_tensor("wqkv", [H, QKV_N], BF16, **ikind)
    if o["wo_res"]:
        wo = nc.dram_tensor("wo", [HQ * HD, H], BF16, **ikind)
    else:
        # mt-major layout: [p, mt, hq, d] -> per-partition-contiguous 1KB
        # slices per mt for efficient streaming DMA
        wo = nc.dram_tensor("wo", [128, MT, HQ, HD], BF16, **ikind)
    cq = nc.dram_tensor("cq", [HD, S], BF16, **ikind)
    sq = nc.dram_tensor("sq", [HD, S], BF16, **ikind)
    ck = nc.dram_tensor("ck", [HD, S], BF16, **ikind)
    sk = nc.dram_tensor("sk", [HD, S], BF16, **ikind)
    if mode == "causal":
        cmask_shape = [HD, 128] if o["offset_scores"] else [HD, QTN, CH]
        cmask = nc.dram_tensor("cmask", cmask_shape, BF16, **ikind)
    if mode == "general":
        maskT = nc.dram_tensor("maskT", [S, S], FP32, **ikind)
    out_dt = BF16 if o["out_bf16"] else FP32
    outT = nc.dram_tensor("outT", [H, S], out_dt, **okind)
    if internal_io:
        dum_in = nc.dram_tensor("dum_in", [1, 8], FP32, kind="ExternalInput")
        dum_out = nc.dram_tensor("dum_out", [1, 8], FP32, kind="ExternalOutput")

    with tile.TileContext(nc) as tc:
        import contextlib
        ctx = contextlib.ExitStack()
        with ctx:
            singles = ctx.enter_context(tc.tile_pool(name="singles", bufs=1))
            small = ctx.enter_context(tc.tile_pool(name="small", bufs=4))
            outp = ctx.enter_context(tc.tile_pool(name="outp", bufs=2))
            wop = ctx.enter_context(tc.tile_pool(name="wop", bufs=4))
            msk = ctx.enter_context(tc.tile_pool(name="msk", bufs=3)) if mode == "general" else None

            def body(_iv=None):
                # ---- resident tensors ----
                wo_sb = None
                if o["wo_res"] and o["proj"] != "nchain":
                    wo_sb = singles.tile([128, HQ, H], BF16, tag="wo_sb",
                                         name="wo_sb")
                    nc.sync.dma_start(
                        out=wo_sb[:],
                        in_=wo.rearrange("(hq p) m -> p hq m", p=128))
                ident = singles.tile([128, 128], BF16, tag="ident", name="ident")
                make_identity(nc, ident[:])
                if mode == "causal":
                    cm_shape = [128, 128] if o["offset_scores"] else [128, QTN, CH]
                    cm_sb = singles.tile(cm_shape, BF16, tag="cm_sb", name="cm_sb")
                    nc.sync.dma_start(out=cm_sb[:], in_=cmask[:])

                qT = singles.tile([128, HQ, S], BF16, tag="qT", name="qT")
                kT = singles.tile([128, S], BF16, tag="kT", name="kT")
                ones128 = None
                if o["pv"] == "flip":
                    ones128 = singles.tile([128, 128], BF16, tag="ones128",
                                           name="ones128")
                    nc.vector.memset(ones128[:], 1.0)
                # inner dim 256 (not 132): DMA-transpose destinations must be
                # 256B-aligned; 132*2B steps corrupt on hardware.
                vtil = singles.tile([128, NKT, 256], BF16, tag="vtil", name="vtil")
                nc.vector.memset(vtil[:, :, 128:129], 1.0)
                attnT = singles.tile([128, HQ, S], BF16, tag="attnT", name="attnT")

                # ---- phase 1: qkv projections (+rope, +v transpose) ----
                # phase-1-only SBUF lives in a nested scope released before
                # attention so the big expP tiles fit.
                import contextlib as _ctl
                with _ctl.ExitStack() as p1:
                    wpool = p1.enter_context(tc.tile_pool(name="wpool", bufs=1))
                    xpool = p1.enter_context(tc.tile_pool(name="xpool", bufs=o["xpool_bufs"]))
                    ropet = p1.enter_context(tc.tile_pool(name="ropet", bufs=2))
                    ropet2 = p1.enter_context(tc.tile_pool(name="ropet2", bufs=2))
                    psA = p1.enter_context(
                        tc.tile_pool(name="psA", bufs=o["psA_bufs"], space="PSUM"))
                    ropestr = p1.enter_context(tc.tile_pool(name="ropestr", bufs=2))
                    wqkv_sb = wpool.tile([128, HT, QKV_N], BF16, tag="wqkv_sb", name="wqkv_sb")
                    wq_r = wqkv.rearrange("(ht p) n -> p ht n", p=128)
                    for hg in range(8):
                        nc.sync.dma_start(
                            out=wqkv_sb[:, hg * (HT // 8):(hg + 1) * (HT // 8), :],
                            in_=wq_r[:, hg * (HT // 8):(hg + 1) * (HT // 8), :])

                    # phases&1 == 0: run a 1/32-depth projection (ht=0 only)
                    # so qT/kT/vtil are still written — timing diagnostic.
                    HTe = HT if (phases & 1) else 1

                    def rope_one(pjt, n, ss, rtile):
                        if n < HQ:
                            cos_t, sin_t = rtile[:, 0, :], rtile[:, 1, :]
                            dst = qT[:, n, ss]
                        else:
                            cos_t, sin_t = rtile[:, 2, :], rtile[:, 3, :]
                            dst = kT[:, ss]
                        if o["rope"] == "copy":
                            nc.vector.tensor_copy(out=dst, in_=pjt[:])
                            return
                        # PSUM-sourced rotate-half multiplies: HW supports
                        # a partition-shifted PSUM read against an SBUF
                        # operand; the all-SBUF variant does not work.
                        t1 = ropet2.tile([128, 512], BF16, tag="t1", name="t1")
                        t2 = ropet2.tile([128, 512], BF16, tag="t2", name="t2")
                        nc.vector.tensor_tensor(t1[:], pjt[:], cos_t, OP.mult)
                        nc.vector.tensor_tensor(
                            t2[0:64, :], pjt[64:128, :], sin_t[0:64], OP.mult)
                        nc.vector.tensor_tensor(
                            t2[64:128, :], pjt[0:64, :], sin_t[64:128], OP.mult)
                        nc.vector.tensor_tensor(dst, t1[:], t2[:], OP.add)

                    def v_one(pjt, sc, j):
                        t = sc * 4 + j
                        vtmp = small.tile([128, 128], BF16, tag="vtmp", name="vtmp")
                        nc.vector.tensor_copy(
                            out=vtmp[:], in_=pjt[:, j * 128:(j + 1) * 128])
                        if o["vtr_dma"]:
                            nc.sync.dma_start(out=vtil[:, t, 0:128],
                                              in_=vtmp[:], transpose=True)
                        else:
                            trp = psA.tile([128, 128], BF16, tag="pst",
                                           name="pst", bufs=o["pstA_bufs"])
                            nc.tensor.transpose(trp[:], vtmp[:], ident[:])
                            nc.vector.tensor_copy(out=vtil[:, t, 0:128],
                                                  in_=trp[:])

                    if o["proj"] == "nchain":
                        # one projection chain at a time: 32 consecutive MMs
                        # into a single PSUM bank (no bank cycling -> no HAM
                        # oscillation), x held resident in SBUF per s-chunk.
                        xt_r = xt.rearrange("(ht p) s -> p ht s", p=128)
                        XG = o["xg"]
                        for sc in range(NSC):
                            ss = slice(sc * 512, (sc + 1) * 512)
                            rtile = ropestr.tile([128, 4, 512], BF16,
                                                 tag="rtile", name="rtile")
                            for ri, t in enumerate((cq, sq, ck, sk)):
                                nc.sync.dma_start(out=rtile[:, ri, :],
                                                  in_=t[:, ss])
                            xres = xpool.tile([128, HT, 512], BF16,
                                              tag="xres", name="xres")
                            for g in range(XG):
                                gs = slice(g * (HT // XG), (g + 1) * (HT // XG))
                                nc.sync.dma_start(out=xres[:, gs, :],
                                                  in_=xt_r[:, gs, ss])
                            for n in range(NQKV):
                                pjt = psA.tile([128, 512], FP32, tag="ps",
                                               name="ps", bufs=o["pj_bufs"])
                                for ht in range(HTe):
                                    nc.tensor.matmul(
                                        pjt[:],
                                        wqkv_sb[:, ht, n * 128:(n + 1) * 128],
                                        xres[:, ht, :],
                                        start=(ht == 0), stop=(ht == HTe - 1))
                                if n < HQ + 1:
                                    rope_one(pjt, n, ss, rtile)
                                else:
                                    for j in range(4):
                                        v_one(pjt, sc, j)
                    else:
                        for sc in range(NSC):
                            ss = slice(sc * 512, (sc + 1) * 512)
                            rtile = ropestr.tile([128, 4, 512], BF16, tag="rtile", name="rtile")
                            for ri, t in enumerate((cq, sq, ck, sk)):
                                nc.sync.dma_start(out=rtile[:, ri, :], in_=t[:, ss])
                            pj = [psA.tile([128, 512], FP32, tag="ps", name="ps") for _ in range(NQKV)]
                            for ht in range(HTe):
                                xtile = xpool.tile([128, 512], BF16, tag="xtile", name="xtile")
                                nc.sync.dma_start(out=xtile[:],
                                                  in_=xt[ht * 128:(ht + 1) * 128, ss])
                                for n in range(NQKV):
                                    nc.tensor.matmul(
                                        pj[n][:],
                                        wqkv_sb[:, ht, n * 128:(n + 1) * 128],
                                        xtile[:],
                                        start=(ht == 0), stop=(ht == HTe - 1))
                            # rope for q heads and k
                            for n in range(HQ + 1):
                                rope_one(pj[n], n, ss, rtile)
                            # v: cast + transpose into vtil
                            for j in range(4):
                                v_one(pj[HQ + 1], sc, j)

                if o["wo_res"] and o["proj"] == "nchain":
                    # loaded after phase-1 SBUF is released (the 48KB wqkv
                    # region is reused); oproj needs it only well into phase 2.
                    woresp = ctx.enter_context(
                        tc.tile_pool(name="woresp", bufs=1))
                    wo_sb = woresp.tile([128, HQ, H], BF16, tag="wo_sb",
                                        name="wo_sb")
                    nc.sync.dma_start(
                        out=wo_sb[:],
                        in_=wo.rearrange("(hq p) m -> p hq m", p=128))

                psB = ctx.enter_context(
                    tc.tile_pool(name="psB", bufs=1, space="PSUM"))
                expp = ctx.enter_context(
                    tc.tile_pool(name="expp", bufs=o["expp_bufs"]))

                # ---- phase 2: attention + o_proj, s_q chunks of CH ----
                def scores_exp(ch, hq, fillers=None):
                    """scores^T + exp for (chunk, head) -> expP tile in SBUF.
                    After each kt, pops one PE-filler closure (oproj chain of
                    the previous chunk) so the in-order PE always has ready
                    work while ACT chews on exps."""
                    kt_hi = (QTN * (ch + 1)) if mode == "causal" else NKT
                    ept = expp.tile([128, NKT, CH], BF16, tag="ept", name="ept")
                    sdt = FP32 if o["sc2_dtype"] == "fp32" else BF16
                    # bf16 scores fit a whole CH in one bank and one matmul
                    MMW = 512 if sdt == FP32 else min(CH, 1024)
                    for kt in range(kt_hi):
                        # offset_scores: columns left of the diagonal block
                        # are all-masked -> skip them in matmul/exp; only the
                        # leading 128 computed cols need the triangle mask.
                        if (mode == "causal" and o["offset_scores"]
                                and kt >= QTN * ch):
                            off = (kt - QTN * ch) * 128
                        else:
                            off = 0
                        sp = psB.tile([128, CH], sdt, tag="sc2", name="sc2",
                                       bufs=o["sc2_bufs"])
                        for u in range(CH // MMW):
                            lo = max(off, u * MMW)
                            hi = (u + 1) * MMW
                            if lo >= hi:
                                continue
                            nc.tensor.matmul(
                                sp[:, lo:hi],
                                kT[:, kt * 128:(kt + 1) * 128],
                                qT[:, hq, ch * CH + lo: ch * CH + hi],
                                start=True, stop=True)
                        if fillers:
                            fillers.popleft()()
                        if mode == "general":
                            mt_sb = msk.tile([128, CH], FP32, tag="mt_sb", name="mt_sb")
                            nc.sync.dma_start(
                                out=mt_sb[:],
                                in_=maskT[kt * 128:(kt + 1) * 128,
                                          ch * CH:(ch + 1) * CH])
                            nc.vector.tensor_tensor(
                                sp[:], sp[:], mt_sb[:], OP.add)
                        if o["pv"] == "flip" and off > 0:
                            # flip-PV matmuls read the full CH width; zero the
                            # skipped all-masked columns so they contribute 0.
                            nc.any.memset(ept[:, kt, 0:off], 0.0)
                        if o["exp_dve"]:
                            nc.vector.tensor_copy(out=ept[:, kt, off:],
                                                  in_=sp[:, off:])
                        else:
                            nc.scalar.activation(ept[:, kt, off:],
                                                 sp[:, off:], AF.Exp)
                        if mode == "causal" and kt >= QTN * ch:
                            if o["offset_scores"]:
                                nc.vector.tensor_tensor(
                                    ept[:, kt, off:off + 128],
                                    ept[:, kt, off:off + 128],
                                    cm_sb[:, 0:128], OP.mult)
                            else:
                                j = kt - QTN * ch
                                nc.vector.tensor_tensor(
                                    ept[:, kt, :], ept[:, kt, :],
                                    cm_sb[:, j, :], OP.mult)
                    return ept

                def pv_block(ch, hq, ept):
                    """PV + normalize + transpose into attnT for (chunk, head)."""
                    if o["no_pv"]:
                        return
                    if o["pv"] == "flip":
                        # attn^T computed directly: psum[d, s_q] = vtil^T @ expP
                        # and a broadcast denominator via an all-ones lhsT;
                        # normalize with reciprocal+mult -> attnT, no transpose.
                        kt_hi = (QTN * (ch + 1)) if mode == "causal" else NKT
                        pvf = psB.tile([128, CH], FP32, tag="pvf", name="pvf",
                                       bufs=o["pvf_bufs"])
                        den = psB.tile([128, CH], FP32, tag="den", name="den",
                                       bufs=o["den_bufs"])
                        for kt in range(kt_hi):
                            nc.tensor.matmul(
                                pvf[:], vtil[:, kt, 0:128], ept[:, kt, :],
                                start=(kt == 0), stop=(kt == kt_hi - 1))
                            nc.tensor.matmul(
                                den[:], ones128[:], ept[:, kt, :],
                                start=(kt == 0), stop=(kt == kt_hi - 1))
                        rec = small.tile([128, CH], FP32, tag="rec", name="rec",
                                         bufs=2)
                        nc.vector.reciprocal(rec[:], den[:])
                        nc.vector.tensor_tensor(
                            attnT[:, hq, ch * CH:(ch + 1) * CH],
                            pvf[:], rec[:], OP.mult)
                        return
                    nbs = []
                    for qt in range(QTN):
                        g = QTN * ch + qt
                        n_kt = (g + 1) if mode == "causal" else NKT
                        pv = psB.tile([128, 512], FP32, tag="pv", name="pv",
                                       bufs=o["pv_bufs"])
                        for kt in range(n_kt):
                            nc.tensor.matmul(
                                pv[:, 0:129],
                                ept[:, kt, qt * 128:(qt + 1) * 128],
                                vtil[:, kt, 0:129],
                                start=(kt == 0), stop=(kt == n_kt - 1))
                        rc = small.tile([128, 1], FP32, tag="rc", name="rc")
                        nc.vector.reciprocal(rc[:], pv[:, 128:129])
                        nb = small.tile([128, 128], BF16, tag="nb", name="nb",
                                        bufs=(QTN + 1) if o["split_tr"] else 4)
                        nc.vector.tensor_scalar_mul(nb[:], pv[:, 0:128], rc[:])
                        if o["tr_dma"]:
                            nc.sync.dma_start(
                                out=attnT[:, hq, ch * CH + qt * 128:
                                          ch * CH + (qt + 1) * 128],
                                in_=nb[:], transpose=True)
                            continue
                        if o["split_tr"]:
                            nbs.append((qt, nb))
                            continue
                        trp = psB.tile([128, 128], BF16, tag="pst2", name="pst2",
                                        bufs=o["pst2_bufs"])
                        nc.tensor.transpose(trp[:], nb[:], ident[:])
                        nc.vector.tensor_copy(
                            out=attnT[:, hq, ch * CH + qt * 128:
                                      ch * CH + (qt + 1) * 128],
                            in_=trp[:])
                    for qt, nb in nbs:
                        trp = psB.tile([128, 128], BF16, tag="pst2", name="pst2",
                                        bufs=o["pst2_bufs"])
                        nc.tensor.transpose(trp[:], nb[:], ident[:])
                        nc.vector.tensor_copy(
                            out=attnT[:, hq, ch * CH + qt * 128:
                                      ch * CH + (qt + 1) * 128],
                            in_=trp[:])

                def oproj_chains(ch, span=1):
                    """Return list of closures, each one mt-chain of
                    oproj over chunks [ch, ch+span)."""
                    if o["no_oproj"]:
                        return []
                    return [
                        (lambda mt=mt: oproj_one(ch, mt, span))
                        for mt in range(MT)
                    ]

                def oproj_one(ch, mt, span=1):
                    if o["wo_res"]:
                        wot = wo_sb[:, :, mt * 128:(mt + 1) * 128]
                    else:
                        wot = wop.tile([128, HQ, 128], BF16, tag="wot",
                                       name="wot")
                        nc.sync.dma_start(out=wot[:], in_=wo[:, mt, :, :])
                    for u in range(SCN * span):
                        ss = slice(ch * CH + u * 512, ch * CH + (u + 1) * 512)
                        op_ps = psB.tile([128, 512], FP32, tag="po",
                                          name="po", bufs=o["po_bufs"])
                        for p in range(HQ):
                            nc.tensor.matmul(
                                op_ps[:],
                                wot[:, p, :],
                                attnT[:, p, ss],
                                start=(p == 0), stop=(p == HQ - 1))
                        ob = outp.tile([128, 512],
                                       BF16 if o["out_bf16"] else FP32,
                                       tag="ob", name="ob")
                        if o["evac_tiny"]:
                            nc.vector.tensor_copy(out=ob[:, 0:8],
                                                  in_=op_ps[:, 0:8])
                        elif o["evac_any"]:
                            nc.any.tensor_copy(out=ob[:], in_=op_ps[:])
                        elif o["evac_pool"]:
                            nc.gpsimd.tensor_copy(out=ob[:], in_=op_ps[:])
                        else:
                            nc.vector.tensor_copy(out=ob[:], in_=op_ps[:])
                        nc.sync.dma_start(
                            out=outT[mt * 128:(mt + 1) * 128, ss], in_=ob[:])

                # software-pipeline heads: scores(i+1) traced before PV(i);
                # previous chunk's oproj chains interleaved per-kt as PE
                # fillers inside scores_exp.
                from collections import deque
                work = [(ch, hq) for ch in range(NCH if (phases & 2) else 0)
                        for hq in range(HQ)]
                pend = None       # (ch, hq, ept) awaiting PV
                fillers = deque()
                if o["wo_pair"]:
                    assert NCH % 2 == 0
                for ch, hq in work:
                    ept = scores_exp(ch, hq, fillers)
                    if pend is not None:
                        pch, phq, pept = pend
                        pv_block(pch, phq, pept)
                        if phq == HQ - 1:
                            # chunk pch attention fully traced -> its oproj
                            # chains may now legally interleave as fillers
                            for f in fillers:  # stragglers of pch-1
                                f()
                            if o["wo_pair"]:
                                fillers = deque(
                                    oproj_chains(pch - 1, span=2)
                                    if pch % 2 == 1 else [])
                            else:
                                fillers = deque(oproj_chains(pch))
                    pend = (ch, hq, ept)
                for f in fillers:
                    f()
                if pend is not None:
                    pch, phq, pept = pend
                    pv_block(pch, phq, pept)
                    if o["wo_pair"]:
                        for f in oproj_chains(NCH - 2, span=2):
                            f()
                    else:
                        for f in oproj_chains(NCH - 1):
                            f()

            if loop_k > 0:
                kw = {}
                if o.get("hints"):
                    ET = mybir.EngineType
                    kw["hint_engines"] = (ET.PE, ET.Activation, ET.DVE,
                                          ET.SP, ET.Pool)
                if o.get("stagger"):
                    kw["staggered_reset"] = True
                with tc.For_i(0, loop_k, 1, **kw) as iv:
                    body(iv)
            else:
                body()

            if internal_io:
                dt = small.tile([1, 8], FP32, tag="dt", name="dt")
                nc.sync.dma_start(out=dt[:], in_=dum_in[:])
                nc.sync.dma_start(out=dum_out[:], in_=dt[:])

    nc.compile()
    return nc


# ---------------- host side ----------------

def rope_tables(S, position_ids, theta=10000.0):
    inv = 1.0 / (theta ** (np.arange(0, HD, 2, dtype=np.float64) / HD))
    t = position_ids.astype(np.float64).reshape(-1)      # [S]
    freqs = np.outer(t, inv)                             # [S, HD/2]
    emb = np.concatenate([freqs, freqs], axis=1)         # [S, HD]
    cos = np.cos(emb).astype(np.float32).T               # [HD, S]
    sin = np.sin(emb).astype(np.float32).T
    return cos, sin


def make_host_inputs(hidden_states, position_ids, Wq, Wk, Wv, Wo, mode, S,
                     wo_res=False):
    scale = 1.0 / math.sqrt(HD)
    cos, sin = rope_tables(S, position_ids)
    sgn = np.ones((HD, 1), np.float32)
    sgn[0:HD // 2] = -1.0
    cqh = (cos * scale).astype(ml_dtypes.bfloat16)
    sqh = (sin * sgn * scale).astype(ml_dtypes.bfloat16)
    ckh = cos.astype(ml_dtypes.bfloat16)
    skh = (sin * sgn).astype(ml_dtypes.bfloat16)

    X = np.ascontiguousarray(hidden_states.reshape(S, H).T).astype(ml_dtypes.bfloat16)

    in_maps = []
    for c in range(NCORES):
        qcols = slice(c * HQ * HD, (c + 1) * HQ * HD)
        kvcols = slice(c * HD, (c + 1) * HD)
        wqkv = np.concatenate(
            [Wq[:, qcols], Wk[:, kvcols], Wv[:, kvcols]], axis=1
        ).astype(ml_dtypes.bfloat16)
        woc = Wo[qcols, :].astype(ml_dtypes.bfloat16)
        if wo_res:
            wo_arr = np.ascontiguousarray(woc)
        else:
            # [hq*128+p, mt*128+d] -> [p, mt, hq, d]
            wo_arr = np.ascontiguousarray(
                woc.reshape(HQ, 128, H // 128, 128).transpose(1, 2, 0, 3))
        m = {
            "xt": X,
            "wqkv": np.ascontiguousarray(wqkv),
            "wo": wo_arr,
            "cq": cqh, "sq": sqh, "ck": ckh, "sk": skh,
        }
        in_maps.append(m)
    return in_maps


def causal_patterns(CH=1024):
    """cmask[kk, j, qq] = 1 if qq >= 128*j + kk else 0 -> [128, CH//128, CH]"""
    QTN = CH // 128
    kk = np.arange(128)[:, None, None]
    j = np.arange(QTN)[None, :, None]
    qq = np.arange(CH)[None, None, :]
    return (qq >= 128 * j + kk).astype(ml_dtypes.bfloat16)


def tri128():
    """triangle mask [128, 128]: 1 if qq >= kk (on/below diagonal)"""
    kk = np.arange(128)[:, None]
    qq = np.arange(128)[None, :]
    return (qq >= kk).astype(ml_dtypes.bfloat16)


def detect_mode(attention_mask):
    am = np.asarray(attention_mask).reshape(attention_mask.shape[-2],
                                            attention_mask.shape[-1])
    if not np.any(am):
        return "none"
    S = am.shape[0]
    tri = np.tril(np.ones((S, S), bool))
    if np.all(am[tri] == 0) and np.all(am[~tri] <= -1e8):
        return "causal"
    return "general"


# ======================================================================
# Harness entry point: kernel(**inputs) -> full output [1, S, H] fp32
# ======================================================================
_NC_CACHE = {}

# best measured configuration (see bench_ablate.py history)
BEST_OPTS = {"proj": "nchain", "xpool_bufs": 2, "wo_res": True,
             "vtr_dma": False, "pj_bufs": 3, "pstA_bufs": 2,
             "pv": "flip", "po_bufs": 2, "sc2_bufs": 3,
             "pvf_bufs": 1, "den_bufs": 1, "wo_pair": False}


def _get_nc(mode):
    if mode not in _NC_CACHE:
        _NC_CACHE[mode] = build_nc(S=2048, mode=mode, loop_k=0,
                                   opts=BEST_OPTS)
    return _NC_CACHE[mode]


def kernel(hidden_states, attention_mask, position_ids, Wq, Wk, Wv, Wo):
    from concourse.bass_utils import run_bass_kernel_spmd

    S = 2048
    hidden_states = np.asarray(hidden_states)
    attention_mask = np.asarray(attention_mask)
    position_ids = np.asarray(position_ids)
    Wq, Wk, Wv, Wo = (np.asarray(a) for a in (Wq, Wk, Wv, Wo))

    mode = detect_mode(attention_mask)
    nc = _get_nc(mode)

    in_maps = make_host_inputs(hidden_states, position_ids,
                               Wq, Wk, Wv, Wo, mode, S,
                               wo_res=BEST_OPTS.get("wo_res", False))
    if mode == "causal":
        cm = tri128()
        for m in in_maps:
            m["cmask"] = cm
    if mode == "general":
        mT = np.ascontiguousarray(
            attention_mask.reshape(S, S).T).astype(np.float32)
        for m in in_maps:
            m["maskT"] = mT

    res = run_bass_kernel_spmd(nc, in_maps, core_ids=list(range(NCORES)))

    acc = np.zeros((H, S), np.float32)
    for c in range(NCORES):
        acc += res.results[c]["outT"].astype(np.float32)
    return np.ascontiguousarray(acc.T).reshape(1, S, H).astype(np.float32)



# revision 22
# speedup vs baseline: 1.1224x; 1.1224x over previous
"""Mistral attention TP-8 Bass kernel builder.

Per-core program (core c of 8):
  inputs (bf16 unless noted):
    xt   [H, S]     hidden_states^T
    wqkv [H, 768]   [Wq_c | Wk_c | Wv_c] columns (4 q heads + 1 kv head)
    wo   [512, H]   Wo rows for this core's heads
    cq,sq,ck,sk [128, S]  rope tables^T (bf16), q-tables pre-scaled by 1/sqrt(128),
                    s-tables sign-baked (rows 0:64 negative)
    cmask [128, 4, 512] (causal mode only) diagonal 0/1 patterns
    maskT [S, S] fp32 (general mode only) additive mask transposed
  output:
    outT [H, S] fp32   partial out^T (host sums over cores, transposes)

Compute (all matmuls bf16, fp32 PSUM accumulation; defaults = tuned config):
  proj (proj="nchain"): x s-chunk resident [128, HT, 512]; one projection
      chain at a time (32 consecutive MMs into a single PSUM bank — avoids
      PSUM-bank-cycling HAM oscillation); rope on q (scaled 1/sqrt(d)), k;
      v transposed via PE (vtr_dma=False: DMA-transposes serialize the DMA
      rings against streaming loads and are much slower in practice)
  scores^T tile = kT-tile (lhsT) x qT  -> psum [s_k 128, s_q 512]; ACT exp -> bf16
  PV (pv="flip"): psum [d 128, s_q 512] = vtil-tile (lhsT) x expP accum over
      kt, plus a parallel all-ones-lhsT chain giving the softmax denominator
      broadcast over all 128 partitions; reciprocal+mult writes attnT [d, s]
      directly — no transposes anywhere in phase 2. Columns left of the
      causal diagonal are memset to 0 so full-width accumulation is exact.
  o_proj: lhsT = resident wo_sb tiles (wo_res=True), rhs = attnT
      -> outT [H, S]; chains interleaved per-kt as PE fillers inside
      scores_exp so the in-order PE never stalls on ACT exp (which runs at
      the 2.3x-errata SBUF-write rate, ~134us total).
"""
import sys
sys.path.insert(0, '/opt/trn_rl_repo')

import math
import numpy as np
import ml_dtypes

import concourse.bass as bass
import concourse.mybir as mybir
import concourse.tile as tile
from concourse import bacc
from concourse.masks import make_identity

FP32 = mybir.dt.float32
BF16 = mybir.dt.bfloat16
AF = mybir.ActivationFunctionType
OP = mybir.AluOpType

H = 4096
HD = 128
NH = 32
NKV = 8
NCORES = 8
HQ = NH // NCORES          # 4 q heads per core
NQKV = HQ + 2              # 6 projection n-tiles (4 q, 1 k, 1 v)
QKV_N = NQKV * HD          # 768


def build_nc(S=2048, mode="none", loop_k=0, internal_io=False, phases=3,
             opts=None):
    """mode: 'none' (no mask), 'causal', 'general'. loop_k>0 wraps body in For_i.

    internal_io=True makes all big tensors Internal DRAM scratch (garbage
    values, zero host transfer) for pure device timing runs; only a tiny
    dummy in/out pair remains external.
    """
    # defaults = best measured configuration, so that timing harnesses that
    # call build_nc() without opts measure the optimized kernel.
    o = dict(psA_bufs=8, pstA_bufs=2, xpool_bufs=2, rope="full", po_bufs=2,
             pv_bufs=2, sc2_bufs=3, pst2_bufs=1, expp_bufs=2,
             sc2_dtype="fp32", split_tr=False, offset_scores=True,
             evac_pool=False, ch=512, tr_dma=True, vtr_dma=False,
             wo_pair=False, proj="nchain", pj_bufs=3, xg=4,
             pv="flip", pvf_bufs=1, den_bufs=1,
             no_oproj=False, no_pv=False, exp_dve=False,
             out_bf16=True, evac_any=False, evac_tiny=False, wo_res=True)
    o.update(opts or {})
    assert S % 512 == 0
    NSC = S // 512            # s chunks (projection phase)
    NKT = S // 128            # s_k tiles
    HT = H // 128             # h tiles (contraction for qkv proj)
    MT = H // 128             # output m tiles
    CH = o["ch"]              # attention s_q chunk
    NCH = S // CH
    QTN = CH // 128
    SCN = CH // 512

    nc = bacc.Bacc("TRN2", target_bir_lowering=False, debug=False,
                   num_devices=NCORES)

    ikind = dict() if internal_io else dict(kind="ExternalInput")
    okind = dict() if internal_io else dict(kind="ExternalOutput")
    xt = nc.dram_tensor("xt", [H, S], BF16, **ikind)
    wqkv = nc.dram_tensor("wqkv", [H, QKV_N], BF16, **ikind)
    if o["wo_res"]:
        wo = nc.dram_tensor("wo", [HQ * HD, H], BF16, **ikind)
    else:
        # mt-major layout: [p, mt, hq, d] -> per-partition-contiguous 1KB
        # slices per mt for efficient streaming DMA
        wo = nc.dram_tensor("wo", [128, MT, HQ, HD], BF16, **ikind)
    cq = nc.dram_tensor("cq", [HD, S], BF16, **ikind)
    sq = nc.dram_tensor("sq", [HD, S], BF16, **ikind)
    ck = nc.dram_tensor("ck", [HD, S], BF16, **ikind)
    sk = nc.dram_tensor("sk", [HD, S], BF16, **ikind)
    if mode == "causal":
        cmask_shape = [HD, 128] if o["offset_scores"] else [HD, QTN, CH]
        cmask = nc.dram_tensor("cmask", cmask_shape, BF16, **ikind)
    if mode == "general":
        maskT = nc.dram_tensor("maskT", [S, S], FP32, **ikind)
    out_dt = BF16 if o["out_bf16"] else FP32
    outT = nc.dram_tensor("outT", [H, S], out_dt, **okind)
    if internal_io:
        dum_in = nc.dram_tensor("dum_in", [1, 8], FP32, kind="ExternalInput")
        dum_out = nc.dram_tensor("dum_out", [1, 8], FP32, kind="ExternalOutput")

    with tile.TileContext(nc) as tc:
        import contextlib
        ctx = contextlib.ExitStack()
        with ctx:
            singles = ctx.enter_context(tc.tile_pool(name="singles", bufs=1))
            small = ctx.enter_context(tc.tile_pool(name="small", bufs=4))
            outp = ctx.enter_context(tc.tile_pool(name="outp", bufs=2))
            wop = ctx.enter_context(tc.tile_pool(name="wop", bufs=4))
            msk = ctx.enter_context(tc.tile_pool(name="msk", bufs=3)) if mode == "general" else None

            def body(_iv=None):
                # ---- resident tensors ----
                wo_sb = None
                if o["wo_res"] and o["proj"] != "nchain":
                    wo_sb = singles.tile([128, HQ, H], BF16, tag="wo_sb",
                                         name="wo_sb")
                    nc.sync.dma_start(
                        out=wo_sb[:],
                        in_=wo.rearrange("(hq p) m -> p hq m", p=128))
                ident = singles.tile([128, 128], BF16, tag="ident", name="ident")
                make_identity(nc, ident[:])
                if mode == "causal":
                    cm_shape = [128, 128] if o["offset_scores"] else [128, QTN, CH]
                    cm_sb = singles.tile(cm_shape, BF16, tag="cm_sb", name="cm_sb")
                    nc.sync.dma_start(out=cm_sb[:], in_=cmask[:])

                qT = singles.tile([128, HQ, S], BF16, tag="qT", name="qT")
                kT = singles.tile([128, S], BF16, tag="kT", name="kT")
                ones128 = None
                if o["pv"] == "flip":
                    ones128 = singles.tile([128, 128], BF16, tag="ones128",
                                           name="ones128")
                    nc.vector.memset(ones128[:], 1.0)
                # inner dim 256 (not 132): DMA-transpose destinations must be
                # 256B-aligned; 132*2B steps corrupt on hardware.
                vtil = singles.tile([128, NKT, 256], BF16, tag="vtil", name="vtil")
                nc.vector.memset(vtil[:, :, 128:129], 1.0)
                attnT = singles.tile([128, HQ, S], BF16, tag="attnT", name="attnT")

                # ---- phase 1: qkv projections (+rope, +v transpose) ----
                # phase-1-only SBUF lives in a nested scope released before
                # attention so the big expP tiles fit.
                import contextlib as _ctl
                with _ctl.ExitStack() as p1:
                    wpool = p1.enter_context(tc.tile_pool(name="wpool", bufs=1))
                    xpool = p1.enter_context(tc.tile_pool(name="xpool", bufs=o["xpool_bufs"]))
                    ropet = p1.enter_context(tc.tile_pool(name="ropet", bufs=2))
                    ropet2 = p1.enter_context(tc.tile_pool(name="ropet2", bufs=2))
                    psA = p1.enter_context(
                        tc.tile_pool(name="psA", bufs=o["psA_bufs"], space="PSUM"))
                    ropestr = p1.enter_context(tc.tile_pool(name="ropestr", bufs=2))
                    wqkv_sb = wpool.tile([128, HT, QKV_N], BF16, tag="wqkv_sb", name="wqkv_sb")
                    wq_r = wqkv.rearrange("(ht p) n -> p ht n", p=128)
                    for hg in range(8):
                        nc.sync.dma_start(
                            out=wqkv_sb[:, hg * (HT // 8):(hg + 1) * (HT // 8), :],
                            in_=wq_r[:, hg * (HT // 8):(hg + 1) * (HT // 8), :])

                    # phases&1 == 0: run a 1/32-depth projection (ht=0 only)
                    # so qT/kT/vtil are still written — timing diagnostic.
                    HTe = HT if (phases & 1) else 1

                    def rope_one(pjt, n, ss, rtile):
                        if n < HQ:
                            cos_t, sin_t = rtile[:, 0, :], rtile[:, 1, :]
                            dst = qT[:, n, ss]
                        else:
                            cos_t, sin_t = rtile[:, 2, :], rtile[:, 3, :]
                            dst = kT[:, ss]
                        if o["rope"] == "copy":
                            nc.vector.tensor_copy(out=dst, in_=pjt[:])
                            return
                        if o["rope"] == "act":
                            # rotate-half staged on the (idle) ACT engine via
                            # partition-shifted PSUM copies; DVE then does two
                            # unshifted 2x-mode multiplies and the add.
                            er = ropet2.tile([128, 512], BF16, tag="er",
                                             name="er")
                            nc.scalar.activation(er[0:64, :], pjt[64:128, :],
                                                 AF.Copy)
                            nc.scalar.activation(er[64:128, :], pjt[0:64, :],
                                                 AF.Copy)
                            t1 = ropet2.tile([128, 512], BF16, tag="t1",
                                             name="t1")
                            t2 = ropet2.tile([128, 512], BF16, tag="t2",
                                             name="t2")
                            nc.vector.tensor_tensor(t1[:], pjt[:], cos_t,
                                                    OP.mult)
                            nc.vector.tensor_tensor(t2[:], er[:], sin_t,
                                                    OP.mult)
                            nc.vector.tensor_tensor(dst, t1[:], t2[:], OP.add)
                            return
                        # PSUM-sourced rotate-half multiplies: HW supports
                        # a partition-shifted PSUM read against an SBUF
                        # operand; the all-SBUF variant does not work.
                        t1 = ropet2.tile([128, 512], BF16, tag="t1", name="t1")
                        t2 = ropet2.tile([128, 512], BF16, tag="t2", name="t2")
                        nc.vector.tensor_tensor(t1[:], pjt[:], cos_t, OP.mult)
                        nc.vector.tensor_tensor(
                            t2[0:64, :], pjt[64:128, :], sin_t[0:64], OP.mult)
                        nc.vector.tensor_tensor(
                            t2[64:128, :], pjt[0:64, :], sin_t[64:128], OP.mult)
                        nc.vector.tensor_tensor(dst, t1[:], t2[:], OP.add)

                    def v_one(pjt, sc, j):
                        t = sc * 4 + j
                        vtmp = small.tile([128, 128], BF16, tag="vtmp", name="vtmp")
                        nc.vector.tensor_copy(
                            out=vtmp[:], in_=pjt[:, j * 128:(j + 1) * 128])
                        if o["vtr_dma"]:
                            nc.sync.dma_start(out=vtil[:, t, 0:128],
                                              in_=vtmp[:], transpose=True)
                        else:
                            trp = psA.tile([128, 128], BF16, tag="pst",
                                           name="pst", bufs=o["pstA_bufs"])
                            nc.tensor.transpose(trp[:], vtmp[:], ident[:])
                            nc.vector.tensor_copy(out=vtil[:, t, 0:128],
                                                  in_=trp[:])

                    if o["proj"] == "nchain":
                        # one projection chain at a time: 32 consecutive MMs
                        # into a single PSUM bank (no bank cycling -> no HAM
                        # oscillation), x held resident in SBUF per s-chunk.
                        xt_r = xt.rearrange("(ht p) s -> p ht s", p=128)
                        XG = o["xg"]
                        for sc in range(NSC):
                            ss = slice(sc * 512, (sc + 1) * 512)
                            rtile = ropestr.tile([128, 4, 512], BF16,
                                                 tag="rtile", name="rtile")
                            for ri, t in enumerate((cq, sq, ck, sk)):
                                nc.sync.dma_start(out=rtile[:, ri, :],
                                                  in_=t[:, ss])
                            xres = xpool.tile([128, HT, 512], BF16,
                                              tag="xres", name="xres")
                            for g in range(XG):
                                gs = slice(g * (HT // XG), (g + 1) * (HT // XG))
                                nc.sync.dma_start(out=xres[:, gs, :],
                                                  in_=xt_r[:, gs, ss])
                            for n in range(NQKV):
                                pjt = psA.tile([128, 512], FP32, tag="ps",
                                               name="ps", bufs=o["pj_bufs"])
                                for ht in range(HTe):
                                    nc.tensor.matmul(
                                        pjt[:],
                                        wqkv_sb[:, ht, n * 128:(n + 1) * 128],
                                        xres[:, ht, :],
                                        start=(ht == 0), stop=(ht == HTe - 1))
                                if n < HQ + 1:
                                    rope_one(pjt, n, ss, rtile)
                                else:
                                    for j in range(4):
                                        v_one(pjt, sc, j)
                    else:
                        for sc in range(NSC):
                            ss = slice(sc * 512, (sc + 1) * 512)
                            rtile = ropestr.tile([128, 4, 512], BF16, tag="rtile", name="rtile")
                            for ri, t in enumerate((cq, sq, ck, sk)):
                                nc.sync.dma_start(out=rtile[:, ri, :], in_=t[:, ss])
                            pj = [psA.tile([128, 512], FP32, tag="ps", name="ps") for _ in range(NQKV)]
                            for ht in range(HTe):
                                xtile = xpool.tile([128, 512], BF16, tag="xtile", name="xtile")
                                nc.sync.dma_start(out=xtile[:],
                                                  in_=xt[ht * 128:(ht + 1) * 128, ss])
                                for n in range(NQKV):
                                    nc.tensor.matmul(
                                        pj[n][:],
                                        wqkv_sb[:, ht, n * 128:(n + 1) * 128],
                                        xtile[:],
                                        start=(ht == 0), stop=(ht == HTe - 1))
                            # rope for q heads and k
                            for n in range(HQ + 1):
                                rope_one(pj[n], n, ss, rtile)
                            # v: cast + transpose into vtil
                            for j in range(4):
                                v_one(pj[HQ + 1], sc, j)

                if o["wo_res"] and o["proj"] == "nchain":
                    # loaded after phase-1 SBUF is released (the 48KB wqkv
                    # region is reused); oproj needs it only well into phase 2.
                    woresp = ctx.enter_context(
                        tc.tile_pool(name="woresp", bufs=1))
                    wo_sb = woresp.tile([128, HQ, H], BF16, tag="wo_sb",
                                        name="wo_sb")
                    nc.sync.dma_start(
                        out=wo_sb[:],
                        in_=wo.rearrange("(hq p) m -> p hq m", p=128))

                psB = ctx.enter_context(
                    tc.tile_pool(name="psB", bufs=1, space="PSUM"))
                expp = ctx.enter_context(
                    tc.tile_pool(name="expp", bufs=o["expp_bufs"]))

                # ---- phase 2: attention + o_proj, s_q chunks of CH ----
                def scores_exp(ch, hq, fillers=None):
                    """scores^T + exp for (chunk, head) -> expP tile in SBUF.
                    After each kt, pops one PE-filler closure (oproj chain of
                    the previous chunk) so the in-order PE always has ready
                    work while ACT chews on exps."""
                    kt_hi = (QTN * (ch + 1)) if mode == "causal" else NKT
                    ept = expp.tile([128, NKT, CH], BF16, tag="ept", name="ept")
                    sdt = FP32 if o["sc2_dtype"] == "fp32" else BF16
                    # bf16 scores fit a whole CH in one bank and one matmul
                    MMW = 512 if sdt == FP32 else min(CH, 1024)
                    for kt in range(kt_hi):
                        # offset_scores: columns left of the diagonal block
                        # are all-masked -> skip them in matmul/exp; only the
                        # leading 128 computed cols need the triangle mask.
                        if (mode == "causal" and o["offset_scores"]
                                and kt >= QTN * ch):
                            off = (kt - QTN * ch) * 128
                        else:
                            off = 0
                        sp = psB.tile([128, CH], sdt, tag="sc2", name="sc2",
                                       bufs=o["sc2_bufs"])
                        for u in range(CH // MMW):
                            lo = max(off, u * MMW)
                            hi = (u + 1) * MMW
                            if lo >= hi:
                                continue
                            nc.tensor.matmul(
                                sp[:, lo:hi],
                                kT[:, kt * 128:(kt + 1) * 128],
                                qT[:, hq, ch * CH + lo: ch * CH + hi],
                                start=True, stop=True)
                        if fillers:
                            fillers.popleft()()
                        if mode == "general":
                            mt_sb = msk.tile([128, CH], FP32, tag="mt_sb", name="mt_sb")
                            nc.sync.dma_start(
                                out=mt_sb[:],
                                in_=maskT[kt * 128:(kt + 1) * 128,
                                          ch * CH:(ch + 1) * CH])
                            nc.vector.tensor_tensor(
                                sp[:], sp[:], mt_sb[:], OP.add)
                        if o["pv"] == "flip" and off > 0:
                            # flip-PV matmuls read the full CH width; zero the
                            # skipped all-masked columns so they contribute 0.
                            nc.any.memset(ept[:, kt, 0:off], 0.0)
                        if o["exp_dve"]:
                            nc.vector.tensor_copy(out=ept[:, kt, off:],
                                                  in_=sp[:, off:])
                        else:
                            nc.scalar.activation(ept[:, kt, off:],
                                                 sp[:, off:], AF.Exp)
                        if mode == "causal" and kt >= QTN * ch:
                            if o["offset_scores"]:
                                nc.vector.tensor_tensor(
                                    ept[:, kt, off:off + 128],
                                    ept[:, kt, off:off + 128],
                                    cm_sb[:, 0:128], OP.mult)
                            else:
                                j = kt - QTN * ch
                                nc.vector.tensor_tensor(
                                    ept[:, kt, :], ept[:, kt, :],
                                    cm_sb[:, j, :], OP.mult)
                    return ept

                def pv_block(ch, hq, ept):
                    """PV + normalize + transpose into attnT for (chunk, head)."""
                    if o["no_pv"]:
                        return
                    if o["pv"] == "flip":
                        # attn^T computed directly: psum[d, s_q] = vtil^T @ expP
                        # and a broadcast denominator via an all-ones lhsT;
                        # normalize with reciprocal+mult -> attnT, no transpose.
                        kt_hi = (QTN * (ch + 1)) if mode == "causal" else NKT
                        pvf = psB.tile([128, CH], FP32, tag="pvf", name="pvf",
                                       bufs=o["pvf_bufs"])
                        den = psB.tile([128, CH], FP32, tag="den", name="den",
                                       bufs=o["den_bufs"])
                        for kt in range(kt_hi):
                            nc.tensor.matmul(
                                pvf[:], vtil[:, kt, 0:128], ept[:, kt, :],
                                start=(kt == 0), stop=(kt == kt_hi - 1))
                            nc.tensor.matmul(
                                den[:], ones128[:], ept[:, kt, :],
                                start=(kt == 0), stop=(kt == kt_hi - 1))
                        rec = small.tile([128, CH], FP32, tag="rec", name="rec",
                                         bufs=2)
                        nc.vector.reciprocal(rec[:], den[:])
                        nc.vector.tensor_tensor(
                            attnT[:, hq, ch * CH:(ch + 1) * CH],
                            pvf[:], rec[:], OP.mult)
                        return
                    nbs = []
                    for qt in range(QTN):
                        g = QTN * ch + qt
                        n_kt = (g + 1) if mode == "causal" else NKT
                        pv = psB.tile([128, 512], FP32, tag="pv", name="pv",
                                       bufs=o["pv_bufs"])
                        for kt in range(n_kt):
                            nc.tensor.matmul(
                                pv[:, 0:129],
                                ept[:, kt, qt * 128:(qt + 1) * 128],
                                vtil[:, kt, 0:129],
                                start=(kt == 0), stop=(kt == n_kt - 1))
                        rc = small.tile([128, 1], FP32, tag="rc", name="rc")
                        nc.vector.reciprocal(rc[:], pv[:, 128:129])
                        nb = small.tile([128, 128], BF16, tag="nb", name="nb",
                                        bufs=(QTN + 1) if o["split_tr"] else 4)
                        nc.vector.tensor_scalar_mul(nb[:], pv[:, 0:128], rc[:])
                        if o["tr_dma"]:
                            nc.sync.dma_start(
                                out=attnT[:, hq, ch * CH + qt * 128:
                                          ch * CH + (qt + 1) * 128],
                                in_=nb[:], transpose=True)
                            continue
                        if o["split_tr"]:
                            nbs.append((qt, nb))
                            continue
                        trp = psB.tile([128, 128], BF16, tag="pst2", name="pst2",
                                        bufs=o["pst2_bufs"])
                        nc.tensor.transpose(trp[:], nb[:], ident[:])
                        nc.vector.tensor_copy(
                            out=attnT[:, hq, ch * CH + qt * 128:
                                      ch * CH + (qt + 1) * 128],
                            in_=trp[:])
                    for qt, nb in nbs:
                        trp = psB.tile([128, 128], BF16, tag="pst2", name="pst2",
                                        bufs=o["pst2_bufs"])
                        nc.tensor.transpose(trp[:], nb[:], ident[:])
                        nc.vector.tensor_copy(
                            out=attnT[:, hq, ch * CH + qt * 128:
                                      ch * CH + (qt + 1) * 128],
                            in_=trp[:])

                def oproj_chains(ch, span=1):
                    """Return list of closures, each one mt-chain of
                    oproj over chunks [ch, ch+span)."""
                    if o["no_oproj"]:
                        return []
                    return [
                        (lambda mt=mt: oproj_one(ch, mt, span))
                        for mt in range(MT)
                    ]

                def oproj_one(ch, mt, span=1):
                    if o["wo_res"]:
                        wot = wo_sb[:, :, mt * 128:(mt + 1) * 128]
                    else:
                        wot = wop.tile([128, HQ, 128], BF16, tag="wot",
                                       name="wot")
                        nc.sync.dma_start(out=wot[:], in_=wo[:, mt, :, :])
                    for u in range(SCN * span):
                        ss = slice(ch * CH + u * 512, ch * CH + (u + 1) * 512)
                        op_ps = psB.tile([128, 512], FP32, tag="po",
                                          name="po", bufs=o["po_bufs"])
                        for p in range(HQ):
                            nc.tensor.matmul(
                                op_ps[:],
                                wot[:, p, :],
                                attnT[:, p, ss],
                                start=(p == 0), stop=(p == HQ - 1))
                        ob = outp.tile([128, 512],
                                       BF16 if o["out_bf16"] else FP32,
                                       tag="ob", name="ob")
                        if o["evac_tiny"]:
                            nc.vector.tensor_copy(out=ob[:, 0:8],
                                                  in_=op_ps[:, 0:8])
                        elif o["evac_any"]:
                            nc.any.tensor_copy(out=ob[:], in_=op_ps[:])
                        elif o["evac_pool"]:
                            nc.gpsimd.tensor_copy(out=ob[:], in_=op_ps[:])
                        elif o.get("evac_alt") and mt % 2 == 1:
                            nc.scalar.activation(ob[:], op_ps[:], AF.Copy)
                        else:
                            nc.vector.tensor_copy(out=ob[:], in_=op_ps[:])
                        nc.sync.dma_start(
                            out=outT[mt * 128:(mt + 1) * 128, ss], in_=ob[:])

                # software-pipeline heads: scores(i+1) traced before PV(i);
                # previous chunk's oproj chains interleaved per-kt as PE
                # fillers inside scores_exp.
                from collections import deque
                work = [(ch, hq) for ch in range(NCH if (phases & 2) else 0)
                        for hq in range(HQ)]
                pend = None       # (ch, hq, ept) awaiting PV
                fillers = deque()
                if o["wo_pair"]:
                    assert NCH % 2 == 0
                for ch, hq in work:
                    ept = scores_exp(ch, hq, fillers)
                    if pend is not None:
                        pch, phq, pept = pend
                        pv_block(pch, phq, pept)
                        if phq == HQ - 1:
                            # chunk pch attention fully traced -> its oproj
                            # chains may now legally interleave as fillers
                            for f in fillers:  # stragglers of pch-1
                                f()
                            if o["wo_pair"]:
                                fillers = deque(
                                    oproj_chains(pch - 1, span=2)
                                    if pch % 2 == 1 else [])
                            else:
                                fillers = deque(oproj_chains(pch))
                    pend = (ch, hq, ept)
                for f in fillers:
                    f()
                if pend is not None:
                    pch, phq, pept = pend
                    pv_block(pch, phq, pept)
                    if o["wo_pair"]:
                        for f in oproj_chains(NCH - 2, span=2):
                            f()
                    else:
                        for f in oproj_chains(NCH - 1):
                            f()

            if loop_k > 0:
                kw = {}
                if o.get("hints"):
                    ET = mybir.EngineType
                    kw["hint_engines"] = (ET.PE, ET.Activation, ET.DVE,
                                          ET.SP, ET.Pool)
                if o.get("stagger"):
                    kw["staggered_reset"] = True
                with tc.For_i(0, loop_k, 1, **kw) as iv:
                    body(iv)
            else:
                body()

            if internal_io:
                dt = small.tile([1, 8], FP32, tag="dt", name="dt")
                nc.sync.dma_start(out=dt[:], in_=dum_in[:])
                nc.sync.dma_start(out=dum_out[:], in_=dt[:])

    nc.compile()
    return nc


# ---------------- host side ----------------

def rope_tables(S, position_ids, theta=10000.0):
    inv = 1.0 / (theta ** (np.arange(0, HD, 2, dtype=np.float64) / HD))
    t = position_ids.astype(np.float64).reshape(-1)      # [S]
    freqs = np.outer(t, inv)                             # [S, HD/2]
    emb = np.concatenate([freqs, freqs], axis=1)         # [S, HD]
    cos = np.cos(emb).astype(np.float32).T               # [HD, S]
    sin = np.sin(emb).astype(np.float32).T
    return cos, sin


def make_host_inputs(hidden_states, position_ids, Wq, Wk, Wv, Wo, mode, S,
                     wo_res=False):
    scale = 1.0 / math.sqrt(HD)
    cos, sin = rope_tables(S, position_ids)
    sgn = np.ones((HD, 1), np.float32)
    sgn[0:HD // 2] = -1.0
    cqh = (cos * scale).astype(ml_dtypes.bfloat16)
    sqh = (sin * sgn * scale).astype(ml_dtypes.bfloat16)
    ckh = cos.astype(ml_dtypes.bfloat16)
    skh = (sin * sgn).astype(ml_dtypes.bfloat16)

    X = np.ascontiguousarray(hidden_states.reshape(S, H).T).astype(ml_dtypes.bfloat16)

    in_maps = []
    for c in range(NCORES):
        qcols = slice(c * HQ * HD, (c + 1) * HQ * HD)
        kvcols = slice(c * HD, (c + 1) * HD)
        wqkv = np.concatenate(
            [Wq[:, qcols], Wk[:, kvcols], Wv[:, kvcols]], axis=1
        ).astype(ml_dtypes.bfloat16)
        woc = Wo[qcols, :].astype(ml_dtypes.bfloat16)
        if wo_res:
            wo_arr = np.ascontiguousarray(woc)
        else:
            # [hq*128+p, mt*128+d] -> [p, mt, hq, d]
            wo_arr = np.ascontiguousarray(
                woc.reshape(HQ, 128, H // 128, 128).transpose(1, 2, 0, 3))
        m = {
            "xt": X,
            "wqkv": np.ascontiguousarray(wqkv),
            "wo": wo_arr,
            "cq": cqh, "sq": sqh, "ck": ckh, "sk": skh,
        }
        in_maps.append(m)
    return in_maps


def causal_patterns(CH=1024):
    """cmask[kk, j, qq] = 1 if qq >= 128*j + kk else 0 -> [128, CH//128, CH]"""
    QTN = CH // 128
    kk = np.arange(128)[:, None, None]
    j = np.arange(QTN)[None, :, None]
    qq = np.arange(CH)[None, None, :]
    return (qq >= 128 * j + kk).astype(ml_dtypes.bfloat16)


def tri128():
    """triangle mask [128, 128]: 1 if qq >= kk (on/below diagonal)"""
    kk = np.arange(128)[:, None]
    qq = np.arange(128)[None, :]
    return (qq >= kk).astype(ml_dtypes.bfloat16)


def detect_mode(attention_mask):
    am = np.asarray(attention_mask).reshape(attention_mask.shape[-2],
                                            attention_mask.shape[-1])
    if not np.any(am):
        return "none"
    S = am.shape[0]
    tri = np.tril(np.ones((S, S), bool))
    if np.all(am[tri] == 0) and np.all(am[~tri] <= -1e8):
        return "causal"
    return "general"


# ======================================================================
# Harness entry point: kernel(**inputs) -> full output [1, S, H] fp32
# ======================================================================
_NC_CACHE = {}

# best measured configuration (see bench_ablate.py history)
BEST_OPTS = {"proj": "nchain", "xpool_bufs": 2, "wo_res": True,
             "vtr_dma": False, "pj_bufs": 3, "pstA_bufs": 2,
             "pv": "flip", "po_bufs": 2, "sc2_bufs": 3,
             "pvf_bufs": 1, "den_bufs": 1, "wo_pair": False}


def _get_nc(mode):
    if mode not in _NC_CACHE:
        _NC_CACHE[mode] = build_nc(S=2048, mode=mode, loop_k=0,
                                   opts=BEST_OPTS)
    return _NC_CACHE[mode]


def kernel(hidden_states, attention_mask, position_ids, Wq, Wk, Wv, Wo):
    from concourse.bass_utils import run_bass_kernel_spmd

    S = 2048
    hidden_states = np.asarray(hidden_states)
    attention_mask = np.asarray(attention_mask)
    position_ids = np.asarray(position_ids)
    Wq, Wk, Wv, Wo = (np.asarray(a) for a in (Wq, Wk, Wv, Wo))

    mode = detect_mode(attention_mask)
    nc = _get_nc(mode)

    in_maps = make_host_inputs(hidden_states, position_ids,
                               Wq, Wk, Wv, Wo, mode, S,
                               wo_res=BEST_OPTS.get("wo_res", False))
    if mode == "causal":
        cm = tri128()
        for m in in_maps:
            m["cmask"] = cm
    if mode == "general":
        mT = np.ascontiguousarray(
            attention_mask.reshape(S, S).T).astype(np.float32)
        for m in in_maps:
            m["maskT"] = mT

    res = run_bass_kernel_spmd(nc, in_maps, core_ids=list(range(NCORES)))

    acc = np.zeros((H, S), np.float32)
    for c in range(NCORES):
        acc += res.results[c]["outT"].astype(np.float32)
    return np.ascontiguousarray(acc.T).reshape(1, S, H).astype(np.float32)



# revision 23
# speedup vs baseline: 1.1462x; 1.0212x over previous
"""Mistral attention TP-8 Bass kernel builder.

Per-core program (core c of 8):
  inputs (bf16 unless noted):
    xt   [H, S]     hidden_states^T
    wqkv [H, 768]   [Wq_c | Wk_c | Wv_c] columns (4 q heads + 1 kv head)
    wo   [512, H]   Wo rows for this core's heads
    cq,sq,ck,sk [128, S]  rope tables^T (bf16), q-tables pre-scaled by 1/sqrt(128),
                    s-tables sign-baked (rows 0:64 negative)
    cmask [128, 4, 512] (causal mode only) diagonal 0/1 patterns
    maskT [S, S] fp32 (general mode only) additive mask transposed
  output:
    outT [H, S] fp32   partial out^T (host sums over cores, transposes)

Compute (all matmuls bf16, fp32 PSUM accumulation; defaults = tuned config):
  proj (proj="nchain"): x s-chunk resident [128, HT, 512]; one projection
      chain at a time (32 consecutive MMs into a single PSUM bank — avoids
      PSUM-bank-cycling HAM oscillation); rope on q (scaled 1/sqrt(d)), k;
      v transposed via PE (vtr_dma=False: DMA-transposes serialize the DMA
      rings against streaming loads and are much slower in practice)
  scores^T tile = kT-tile (lhsT) x qT  -> psum [s_k 128, s_q 512]; ACT exp -> bf16
  PV (pv="flip"): psum [d 128, s_q 512] = vtil-tile (lhsT) x expP accum over
      kt, plus a parallel all-ones-lhsT chain giving the softmax denominator
      broadcast over all 128 partitions; reciprocal+mult writes attnT [d, s]
      directly — no transposes anywhere in phase 2. Columns left of the
      causal diagonal are memset to 0 so full-width accumulation is exact.
  o_proj: lhsT = resident wo_sb tiles (wo_res=True), rhs = attnT
      -> outT [H, S]; chains interleaved per-kt as PE fillers inside
      scores_exp so the in-order PE never stalls on ACT exp (which runs at
      the 2.3x-errata SBUF-write rate, ~134us total).
"""
import sys
sys.path.insert(0, '/opt/trn_rl_repo')

import math
import numpy as np
import ml_dtypes

import concourse.bass as bass
import concourse.mybir as mybir
import concourse.tile as tile
from concourse import bacc
from concourse.masks import make_identity

FP32 = mybir.dt.float32
BF16 = mybir.dt.bfloat16
AF = mybir.ActivationFunctionType
OP = mybir.AluOpType

H = 4096
HD = 128
NH = 32
NKV = 8
NCORES = 8
HQ = NH // NCORES          # 4 q heads per core
NQKV = HQ + 2              # 6 projection n-tiles (4 q, 1 k, 1 v)
QKV_N = NQKV * HD          # 768


def build_nc(S=2048, mode="none", loop_k=0, internal_io=False, phases=3,
             opts=None):
    """mode: 'none' (no mask), 'causal', 'general'. loop_k>0 wraps body in For_i.

    internal_io=True makes all big tensors Internal DRAM scratch (garbage
    values, zero host transfer) for pure device timing runs; only a tiny
    dummy in/out pair remains external.
    """
    # defaults = best measured configuration, so that timing harnesses that
    # call build_nc() without opts measure the optimized kernel.
    o = dict(psA_bufs=8, pstA_bufs=2, xpool_bufs=2, rope="full", po_bufs=2,
             pv_bufs=2, sc2_bufs=3, pst2_bufs=1, expp_bufs=2,
             sc2_dtype="fp32", split_tr=False, offset_scores=True,
             evac_pool=False, ch=512, tr_dma=True, vtr_dma=False,
             wo_pair=False, proj="nchain", pj_bufs=3, xg=4,
             pv="flip", pvf_bufs=2, den_bufs=1,
             no_oproj=False, no_pv=False, exp_dve=False,
             out_bf16=True, evac_any=False, evac_tiny=False, wo_res=True)
    o.update(opts or {})
    assert S % 512 == 0
    NSC = S // 512            # s chunks (projection phase)
    NKT = S // 128            # s_k tiles
    HT = H // 128             # h tiles (contraction for qkv proj)
    MT = H // 128             # output m tiles
    CH = o["ch"]              # attention s_q chunk
    NCH = S // CH
    QTN = CH // 128
    SCN = CH // 512

    nc = bacc.Bacc("TRN2", target_bir_lowering=False, debug=False,
                   num_devices=NCORES)

    ikind = dict() if internal_io else dict(kind="ExternalInput")
    okind = dict() if internal_io else dict(kind="ExternalOutput")
    xt = nc.dram_tensor("xt", [H, S], BF16, **ikind)
    wqkv = nc.dram_tensor("wqkv", [H, QKV_N], BF16, **ikind)
    if o["wo_res"]:
        wo = nc.dram_tensor("wo", [HQ * HD, H], BF16, **ikind)
    else:
        # mt-major layout: [p, mt, hq, d] -> per-partition-contiguous 1KB
        # slices per mt for efficient streaming DMA
        wo = nc.dram_tensor("wo", [128, MT, HQ, HD], BF16, **ikind)
    cq = nc.dram_tensor("cq", [HD, S], BF16, **ikind)
    sq = nc.dram_tensor("sq", [HD, S], BF16, **ikind)
    ck = nc.dram_tensor("ck", [HD, S], BF16, **ikind)
    sk = nc.dram_tensor("sk", [HD, S], BF16, **ikind)
    if mode == "causal":
        cmask_shape = [HD, 128] if o["offset_scores"] else [HD, QTN, CH]
        cmask = nc.dram_tensor("cmask", cmask_shape, BF16, **ikind)
    if mode == "general":
        maskT = nc.dram_tensor("maskT", [S, S], FP32, **ikind)
    out_dt = BF16 if o["out_bf16"] else FP32
    outT = nc.dram_tensor("outT", [H, S], out_dt, **okind)
    if internal_io:
        dum_in = nc.dram_tensor("dum_in", [1, 8], FP32, kind="ExternalInput")
        dum_out = nc.dram_tensor("dum_out", [1, 8], FP32, kind="ExternalOutput")

    with tile.TileContext(nc) as tc:
        import contextlib
        ctx = contextlib.ExitStack()
        with ctx:
            singles = ctx.enter_context(tc.tile_pool(name="singles", bufs=1))
            small = ctx.enter_context(tc.tile_pool(name="small", bufs=4))
            outp = ctx.enter_context(tc.tile_pool(name="outp", bufs=2))
            wop = ctx.enter_context(tc.tile_pool(name="wop", bufs=4))
            msk = ctx.enter_context(tc.tile_pool(name="msk", bufs=3)) if mode == "general" else None

            def body(_iv=None):
                # ---- resident tensors ----
                wo_sb = None
                if o["wo_res"] and o["proj"] != "nchain":
                    wo_sb = singles.tile([128, HQ, H], BF16, tag="wo_sb",
                                         name="wo_sb")
                    nc.sync.dma_start(
                        out=wo_sb[:],
                        in_=wo.rearrange("(hq p) m -> p hq m", p=128))
                ident = singles.tile([128, 128], BF16, tag="ident", name="ident")
                make_identity(nc, ident[:])
                if mode == "causal":
                    cm_shape = [128, 128] if o["offset_scores"] else [128, QTN, CH]
                    cm_sb = singles.tile(cm_shape, BF16, tag="cm_sb", name="cm_sb")
                    nc.sync.dma_start(out=cm_sb[:], in_=cmask[:])

                qT = singles.tile([128, HQ, S], BF16, tag="qT", name="qT")
                kT = singles.tile([128, S], BF16, tag="kT", name="kT")
                ones128 = None
                if o["pv"] == "flip":
                    ones128 = singles.tile([128, 128], BF16, tag="ones128",
                                           name="ones128")
                    nc.vector.memset(ones128[:], 1.0)
                # inner dim 256 (not 132): DMA-transpose destinations must be
                # 256B-aligned; 132*2B steps corrupt on hardware.
                vtil = singles.tile([128, NKT, 256], BF16, tag="vtil", name="vtil")
                nc.vector.memset(vtil[:, :, 128:129], 1.0)
                attnT = singles.tile([128, HQ, S], BF16, tag="attnT", name="attnT")

                # ---- phase 1: qkv projections (+rope, +v transpose) ----
                # phase-1-only SBUF lives in a nested scope released before
                # attention so the big expP tiles fit.
                import contextlib as _ctl
                with _ctl.ExitStack() as p1:
                    wpool = p1.enter_context(tc.tile_pool(name="wpool", bufs=1))
                    xpool = p1.enter_context(tc.tile_pool(name="xpool", bufs=o["xpool_bufs"]))
                    ropet = p1.enter_context(tc.tile_pool(name="ropet", bufs=2))
                    ropet2 = p1.enter_context(tc.tile_pool(name="ropet2", bufs=2))
                    psA = p1.enter_context(
                        tc.tile_pool(name="psA", bufs=o["psA_bufs"], space="PSUM"))
                    ropestr = p1.enter_context(tc.tile_pool(name="ropestr", bufs=2))
                    wqkv_sb = wpool.tile([128, HT, QKV_N], BF16, tag="wqkv_sb", name="wqkv_sb")
                    wq_r = wqkv.rearrange("(ht p) n -> p ht n", p=128)
                    for hg in range(8):
                        nc.sync.dma_start(
                            out=wqkv_sb[:, hg * (HT // 8):(hg + 1) * (HT // 8), :],
                            in_=wq_r[:, hg * (HT // 8):(hg + 1) * (HT // 8), :])

                    # phases&1 == 0: run a 1/32-depth projection (ht=0 only)
                    # so qT/kT/vtil are still written — timing diagnostic.
                    HTe = HT if (phases & 1) else 1

                    def rope_one(pjt, n, ss, rtile):
                        if n < HQ:
                            cos_t, sin_t = rtile[:, 0, :], rtile[:, 1, :]
                            dst = qT[:, n, ss]
                        else:
                            cos_t, sin_t = rtile[:, 2, :], rtile[:, 3, :]
                            dst = kT[:, ss]
                        if o["rope"] == "copy":
                            nc.vector.tensor_copy(out=dst, in_=pjt[:])
                            return
                        if o["rope"] == "act":
                            # rotate-half staged on the (idle) ACT engine via
                            # partition-shifted PSUM copies; DVE then does two
                            # unshifted 2x-mode multiplies and the add.
                            er = ropet2.tile([128, 512], BF16, tag="er",
                                             name="er")
                            nc.scalar.activation(er[0:64, :], pjt[64:128, :],
                                                 AF.Copy)
                            nc.scalar.activation(er[64:128, :], pjt[0:64, :],
                                                 AF.Copy)
                            t1 = ropet2.tile([128, 512], BF16, tag="t1",
                                             name="t1")
                            t2 = ropet2.tile([128, 512], BF16, tag="t2",
                                             name="t2")
                            nc.vector.tensor_tensor(t1[:], pjt[:], cos_t,
                                                    OP.mult)
                            nc.vector.tensor_tensor(t2[:], er[:], sin_t,
                                                    OP.mult)
                            nc.vector.tensor_tensor(dst, t1[:], t2[:], OP.add)
                            return
                        # PSUM-sourced rotate-half multiplies: HW supports
                        # a partition-shifted PSUM read against an SBUF
                        # operand; the all-SBUF variant does not work.
                        t1 = ropet2.tile([128, 512], BF16, tag="t1", name="t1")
                        t2 = ropet2.tile([128, 512], BF16, tag="t2", name="t2")
                        nc.vector.tensor_tensor(t1[:], pjt[:], cos_t, OP.mult)
                        nc.vector.tensor_tensor(
                            t2[0:64, :], pjt[64:128, :], sin_t[0:64], OP.mult)
                        nc.vector.tensor_tensor(
                            t2[64:128, :], pjt[0:64, :], sin_t[64:128], OP.mult)
                        nc.vector.tensor_tensor(dst, t1[:], t2[:], OP.add)

                    def v_one(pjt, sc, j):
                        t = sc * 4 + j
                        vtmp = small.tile([128, 128], BF16, tag="vtmp", name="vtmp")
                        nc.vector.tensor_copy(
                            out=vtmp[:], in_=pjt[:, j * 128:(j + 1) * 128])
                        if o["vtr_dma"]:
                            nc.sync.dma_start(out=vtil[:, t, 0:128],
                                              in_=vtmp[:], transpose=True)
                        else:
                            trp = psA.tile([128, 128], BF16, tag="pst",
                                           name="pst", bufs=o["pstA_bufs"])
                            nc.tensor.transpose(trp[:], vtmp[:], ident[:])
                            nc.vector.tensor_copy(out=vtil[:, t, 0:128],
                                                  in_=trp[:])

                    if o["proj"] == "nchain":
                        # one projection chain at a time: 32 consecutive MMs
                        # into a single PSUM bank (no bank cycling -> no HAM
                        # oscillation), x held resident in SBUF per s-chunk.
                        xt_r = xt.rearrange("(ht p) s -> p ht s", p=128)
                        XG = o["xg"]
                        for sc in range(NSC):
                            ss = slice(sc * 512, (sc + 1) * 512)
                            rtile = ropestr.tile([128, 4, 512], BF16,
                                                 tag="rtile", name="rtile")
                            for ri, t in enumerate((cq, sq, ck, sk)):
                                nc.sync.dma_start(out=rtile[:, ri, :],
                                                  in_=t[:, ss])
                            xres = xpool.tile([128, HT, 512], BF16,
                                              tag="xres", name="xres")
                            for g in range(XG):
                                gs = slice(g * (HT // XG), (g + 1) * (HT // XG))
                                nc.sync.dma_start(out=xres[:, gs, :],
                                                  in_=xt_r[:, gs, ss])
                            for n in range(NQKV):
                                pjt = psA.tile([128, 512], FP32, tag="ps",
                                               name="ps", bufs=o["pj_bufs"])
                                for ht in range(HTe):
                                    nc.tensor.matmul(
                                        pjt[:],
                                        wqkv_sb[:, ht, n * 128:(n + 1) * 128],
                                        xres[:, ht, :],
                                        start=(ht == 0), stop=(ht == HTe - 1))
                                if n < HQ + 1:
                                    rope_one(pjt, n, ss, rtile)
                                else:
                                    for j in range(4):
                                        v_one(pjt, sc, j)
                    else:
                        for sc in range(NSC):
                            ss = slice(sc * 512, (sc + 1) * 512)
                            rtile = ropestr.tile([128, 4, 512], BF16, tag="rtile", name="rtile")
                            for ri, t in enumerate((cq, sq, ck, sk)):
                                nc.sync.dma_start(out=rtile[:, ri, :], in_=t[:, ss])
                            pj = [psA.tile([128, 512], FP32, tag="ps", name="ps") for _ in range(NQKV)]
                            for ht in range(HTe):
                                xtile = xpool.tile([128, 512], BF16, tag="xtile", name="xtile")
                                nc.sync.dma_start(out=xtile[:],
                                                  in_=xt[ht * 128:(ht + 1) * 128, ss])
                                for n in range(NQKV):
                                    nc.tensor.matmul(
                                        pj[n][:],
                                        wqkv_sb[:, ht, n * 128:(n + 1) * 128],
                                        xtile[:],
                                        start=(ht == 0), stop=(ht == HTe - 1))
                            # rope for q heads and k
                            for n in range(HQ + 1):
                                rope_one(pj[n], n, ss, rtile)
                            # v: cast + transpose into vtil
                            for j in range(4):
                                v_one(pj[HQ + 1], sc, j)

                if o["wo_res"] and o["proj"] == "nchain":
                    # loaded after phase-1 SBUF is released (the 48KB wqkv
                    # region is reused); oproj needs it only well into phase 2.
                    woresp = ctx.enter_context(
                        tc.tile_pool(name="woresp", bufs=1))
                    wo_sb = woresp.tile([128, HQ, H], BF16, tag="wo_sb",
                                        name="wo_sb")
                    nc.sync.dma_start(
                        out=wo_sb[:],
                        in_=wo.rearrange("(hq p) m -> p hq m", p=128))

                psB = ctx.enter_context(
                    tc.tile_pool(name="psB", bufs=1, space="PSUM"))
                expp = ctx.enter_context(
                    tc.tile_pool(name="expp", bufs=o["expp_bufs"]))

                # ---- phase 2: attention + o_proj, s_q chunks of CH ----
                def scores_exp(ch, hq, fillers=None):
                    """scores^T + exp for (chunk, head) -> expP tile in SBUF.
                    After each kt, pops one PE-filler closure (oproj chain of
                    the previous chunk) so the in-order PE always has ready
                    work while ACT chews on exps."""
                    kt_hi = (QTN * (ch + 1)) if mode == "causal" else NKT
                    ept = expp.tile([128, NKT, CH], BF16, tag="ept", name="ept")
                    sdt = FP32 if o["sc2_dtype"] == "fp32" else BF16
                    # bf16 scores fit a whole CH in one bank and one matmul
                    MMW = 512 if sdt == FP32 else min(CH, 1024)
                    for kt in range(kt_hi):
                        # offset_scores: columns left of the diagonal block
                        # are all-masked -> skip them in matmul/exp; only the
                        # leading 128 computed cols need the triangle mask.
                        if (mode == "causal" and o["offset_scores"]
                                and kt >= QTN * ch):
                            off = (kt - QTN * ch) * 128
                        else:
                            off = 0
                        sp = psB.tile([128, CH], sdt, tag="sc2", name="sc2",
                                       bufs=o["sc2_bufs"])
                        for u in range(CH // MMW):
                            lo = max(off, u * MMW)
                            hi = (u + 1) * MMW
                            if lo >= hi:
                                continue
                            nc.tensor.matmul(
                                sp[:, lo:hi],
                                kT[:, kt * 128:(kt + 1) * 128],
                                qT[:, hq, ch * CH + lo: ch * CH + hi],
                                start=True, stop=True)
                        if fillers:
                            fillers.popleft()()
                        if mode == "general":
                            mt_sb = msk.tile([128, CH], FP32, tag="mt_sb", name="mt_sb")
                            nc.sync.dma_start(
                                out=mt_sb[:],
                                in_=maskT[kt * 128:(kt + 1) * 128,
                                          ch * CH:(ch + 1) * CH])
                            nc.vector.tensor_tensor(
                                sp[:], sp[:], mt_sb[:], OP.add)
                        if o["pv"] == "flip" and off > 0:
                            # flip-PV matmuls read the full CH width; zero the
                            # skipped all-masked columns so they contribute 0.
                            nc.any.memset(ept[:, kt, 0:off], 0.0)
                        if o["exp_dve"]:
                            nc.vector.tensor_copy(out=ept[:, kt, off:],
                                                  in_=sp[:, off:])
                        else:
                            nc.scalar.activation(ept[:, kt, off:],
                                                 sp[:, off:], AF.Exp)
                        if mode == "causal" and kt >= QTN * ch:
                            if o["offset_scores"]:
                                nc.vector.tensor_tensor(
                                    ept[:, kt, off:off + 128],
                                    ept[:, kt, off:off + 128],
                                    cm_sb[:, 0:128], OP.mult)
                            else:
                                j = kt - QTN * ch
                                nc.vector.tensor_tensor(
                                    ept[:, kt, :], ept[:, kt, :],
                                    cm_sb[:, j, :], OP.mult)
                    return ept

                def pv_block(ch, hq, ept):
                    """PV + normalize + transpose into attnT for (chunk, head)."""
                    if o["no_pv"]:
                        return
                    if o["pv"] == "flip":
                        # attn^T computed directly: psum[d, s_q] = vtil^T @ expP
                        # and a broadcast denominator via an all-ones lhsT;
                        # normalize with reciprocal+mult -> attnT, no transpose.
                        kt_hi = (QTN * (ch + 1)) if mode == "causal" else NKT
                        pvf = psB.tile([128, CH], FP32, tag="pvf", name="pvf",
                                       bufs=o["pvf_bufs"])
                        den = psB.tile([128, CH], FP32, tag="den", name="den",
                                       bufs=o["den_bufs"])
                        for kt in range(kt_hi):
                            nc.tensor.matmul(
                                pvf[:], vtil[:, kt, 0:128], ept[:, kt, :],
                                start=(kt == 0), stop=(kt == kt_hi - 1))
                            nc.tensor.matmul(
                                den[:], ones128[:], ept[:, kt, :],
                                start=(kt == 0), stop=(kt == kt_hi - 1))
                        rec = small.tile([128, CH], FP32, tag="rec", name="rec",
                                         bufs=2)
                        nc.vector.reciprocal(rec[:], den[:])
                        nc.vector.tensor_tensor(
                            attnT[:, hq, ch * CH:(ch + 1) * CH],
                            pvf[:], rec[:], OP.mult)
                        return
                    nbs = []
                    for qt in range(QTN):
                        g = QTN * ch + qt
                        n_kt = (g + 1) if mode == "causal" else NKT
                        pv = psB.tile([128, 512], FP32, tag="pv", name="pv",
                                       bufs=o["pv_bufs"])
                        for kt in range(n_kt):
                            nc.tensor.matmul(
                                pv[:, 0:129],
                                ept[:, kt, qt * 128:(qt + 1) * 128],
                                vtil[:, kt, 0:129],
                                start=(kt == 0), stop=(kt == n_kt - 1))
                        rc = small.tile([128, 1], FP32, tag="rc", name="rc")
                        nc.vector.reciprocal(rc[:], pv[:, 128:129])
                        nb = small.tile([128, 128], BF16, tag="nb", name="nb",
                                        bufs=(QTN + 1) if o["split_tr"] else 4)
                        nc.vector.tensor_scalar_mul(nb[:], pv[:, 0:128], rc[:])
                        if o["tr_dma"]:
                            nc.sync.dma_start(
                                out=attnT[:, hq, ch * CH + qt * 128:
                                          ch * CH + (qt + 1) * 128],
                                in_=nb[:], transpose=True)
                            continue
                        if o["split_tr"]:
                            nbs.append((qt, nb))
                            continue
                        trp = psB.tile([128, 128], BF16, tag="pst2", name="pst2",
                                        bufs=o["pst2_bufs"])
                        nc.tensor.transpose(trp[:], nb[:], ident[:])
                        nc.vector.tensor_copy(
                            out=attnT[:, hq, ch * CH + qt * 128:
                                      ch * CH + (qt + 1) * 128],
                            in_=trp[:])
                    for qt, nb in nbs:
                        trp = psB.tile([128, 128], BF16, tag="pst2", name="pst2",
                                        bufs=o["pst2_bufs"])
                        nc.tensor.transpose(trp[:], nb[:], ident[:])
                        nc.vector.tensor_copy(
                            out=attnT[:, hq, ch * CH + qt * 128:
                                      ch * CH + (qt + 1) * 128],
                            in_=trp[:])

                def oproj_chains(ch, span=1):
                    """Return list of closures, each one mt-chain of
                    oproj over chunks [ch, ch+span)."""
                    if o["no_oproj"]:
                        return []
                    return [
                        (lambda mt=mt: oproj_one(ch, mt, span))
                        for mt in range(MT)
                    ]

                def oproj_one(ch, mt, span=1):
                    if o["wo_res"]:
                        wot = wo_sb[:, :, mt * 128:(mt + 1) * 128]
                    else:
                        wot = wop.tile([128, HQ, 128], BF16, tag="wot",
                                       name="wot")
                        nc.sync.dma_start(out=wot[:], in_=wo[:, mt, :, :])
                    for u in range(SCN * span):
                        ss = slice(ch * CH + u * 512, ch * CH + (u + 1) * 512)
                        op_ps = psB.tile([128, 512], FP32, tag="po",
                                          name="po", bufs=o["po_bufs"])
                        for p in range(HQ):
                            nc.tensor.matmul(
                                op_ps[:],
                                wot[:, p, :],
                                attnT[:, p, ss],
                                start=(p == 0), stop=(p == HQ - 1))
                        ob = outp.tile([128, 512],
                                       BF16 if o["out_bf16"] else FP32,
                                       tag="ob", name="ob")
                        if o["evac_tiny"]:
                            nc.vector.tensor_copy(out=ob[:, 0:8],
                                                  in_=op_ps[:, 0:8])
                        elif o["evac_any"]:
                            nc.any.tensor_copy(out=ob[:], in_=op_ps[:])
                        elif o["evac_pool"]:
                            nc.gpsimd.tensor_copy(out=ob[:], in_=op_ps[:])
                        elif o.get("evac_alt") and mt % 2 == 1:
                            nc.scalar.activation(ob[:], op_ps[:], AF.Copy)
                        else:
                            nc.vector.tensor_copy(out=ob[:], in_=op_ps[:])
                        nc.sync.dma_start(
                            out=outT[mt * 128:(mt + 1) * 128, ss], in_=ob[:])

                # software-pipeline heads: scores(i+1) traced before PV(i);
                # previous chunk's oproj chains interleaved per-kt as PE
                # fillers inside scores_exp.
                from collections import deque
                work = [(ch, hq) for ch in range(NCH if (phases & 2) else 0)
                        for hq in range(HQ)]
                pend = None       # (ch, hq, ept) awaiting PV
                fillers = deque()
                if o["wo_pair"]:
                    assert NCH % 2 == 0
                for ch, hq in work:
                    ept = scores_exp(ch, hq, fillers)
                    if pend is not None:
                        pch, phq, pept = pend
                        pv_block(pch, phq, pept)
                        if phq == HQ - 1:
                            # chunk pch attention fully traced -> its oproj
                            # chains may now legally interleave as fillers
                            for f in fillers:  # stragglers of pch-1
                                f()
                            if o["wo_pair"]:
                                fillers = deque(
                                    oproj_chains(pch - 1, span=2)
                                    if pch % 2 == 1 else [])
                            else:
                                fillers = deque(oproj_chains(pch))
                    pend = (ch, hq, ept)
                for f in fillers:
                    f()
                if pend is not None:
                    pch, phq, pept = pend
                    pv_block(pch, phq, pept)
                    if o["wo_pair"]:
                        for f in oproj_chains(NCH - 2, span=2):
                            f()
                    else:
                        for f in oproj_chains(NCH - 1):
                            f()

            if loop_k > 0:
                kw = {}
                if o.get("hints"):
                    ET = mybir.EngineType
                    kw["hint_engines"] = (ET.PE, ET.Activation, ET.DVE,
                                          ET.SP, ET.Pool)
                if o.get("stagger"):
                    kw["staggered_reset"] = True
                with tc.For_i(0, loop_k, 1, **kw) as iv:
                    body(iv)
            else:
                body()

            if internal_io:
                dt = small.tile([1, 8], FP32, tag="dt", name="dt")
                nc.sync.dma_start(out=dt[:], in_=dum_in[:])
                nc.sync.dma_start(out=dum_out[:], in_=dt[:])

    nc.compile()
    return nc


# ---------------- host side ----------------

def rope_tables(S, position_ids, theta=10000.0):
    inv = 1.0 / (theta ** (np.arange(0, HD, 2, dtype=np.float64) / HD))
    t = position_ids.astype(np.float64).reshape(-1)      # [S]
    freqs = np.outer(t, inv)                             # [S, HD/2]
    emb = np.concatenate([freqs, freqs], axis=1)         # [S, HD]
    cos = np.cos(emb).astype(np.float32).T               # [HD, S]
    sin = np.sin(emb).astype(np.float32).T
    return cos, sin


def make_host_inputs(hidden_states, position_ids, Wq, Wk, Wv, Wo, mode, S,
                     wo_res=False):
    scale = 1.0 / math.sqrt(HD)
    cos, sin = rope_tables(S, position_ids)
    sgn = np.ones((HD, 1), np.float32)
    sgn[0:HD // 2] = -1.0
    cqh = (cos * scale).astype(ml_dtypes.bfloat16)
    sqh = (sin * sgn * scale).astype(ml_dtypes.bfloat16)
    ckh = cos.astype(ml_dtypes.bfloat16)
    skh = (sin * sgn).astype(ml_dtypes.bfloat16)

    X = np.ascontiguousarray(hidden_states.reshape(S, H).T).astype(ml_dtypes.bfloat16)

    in_maps = []
    for c in range(NCORES):
        qcols = slice(c * HQ * HD, (c + 1) * HQ * HD)
        kvcols = slice(c * HD, (c + 1) * HD)
        wqkv = np.concatenate(
            [Wq[:, qcols], Wk[:, kvcols], Wv[:, kvcols]], axis=1
        ).astype(ml_dtypes.bfloat16)
        woc = Wo[qcols, :].astype(ml_dtypes.bfloat16)
        if wo_res:
            wo_arr = np.ascontiguousarray(woc)
        else:
            # [hq*128+p, mt*128+d] -> [p, mt, hq, d]
            wo_arr = np.ascontiguousarray(
                woc.reshape(HQ, 128, H // 128, 128).transpose(1, 2, 0, 3))
        m = {
            "xt": X,
            "wqkv": np.ascontiguousarray(wqkv),
            "wo": wo_arr,
            "cq": cqh, "sq": sqh, "ck": ckh, "sk": skh,
        }
        in_maps.append(m)
    return in_maps


def causal_patterns(CH=1024):
    """cmask[kk, j, qq] = 1 if qq >= 128*j + kk else 0 -> [128, CH//128, CH]"""
    QTN = CH // 128
    kk = np.arange(128)[:, None, None]
    j = np.arange(QTN)[None, :, None]
    qq = np.arange(CH)[None, None, :]
    return (qq >= 128 * j + kk).astype(ml_dtypes.bfloat16)


def tri128():
    """triangle mask [128, 128]: 1 if qq >= kk (on/below diagonal)"""
    kk = np.arange(128)[:, None]
    qq = np.arange(128)[None, :]
    return (qq >= kk).astype(ml_dtypes.bfloat16)


def detect_mode(attention_mask):
    am = np.asarray(attention_mask).reshape(attention_mask.shape[-2],
                                            attention_mask.shape[-1])
    if not np.any(am):
        return "none"
    S = am.shape[0]
    tri = np.tril(np.ones((S, S), bool))
    if np.all(am[tri] == 0) and np.all(am[~tri] <= -1e8):
        return "causal"
    return "general"


# ======================================================================
# Harness entry point: kernel(**inputs) -> full output [1, S, H] fp32
# ======================================================================
_NC_CACHE = {}

# best measured configuration (see bench_ablate.py history)
BEST_OPTS = {"proj": "nchain", "xpool_bufs": 2, "wo_res": True,
             "vtr_dma": False, "pj_bufs": 3, "pstA_bufs": 2,
             "pv": "flip", "po_bufs": 2, "sc2_bufs": 3,
             "pvf_bufs": 2, "den_bufs": 1, "wo_pair": False}


def _get_nc(mode):
    if mode not in _NC_CACHE:
        _NC_CACHE[mode] = build_nc(S=2048, mode=mode, loop_k=0,
                                   opts=BEST_OPTS)
    return _NC_CACHE[mode]


def kernel(hidden_states, attention_mask, position_ids, Wq, Wk, Wv, Wo):
    from concourse.bass_utils import run_bass_kernel_spmd

    S = 2048
    hidden_states = np.asarray(hidden_states)
    attention_mask = np.asarray(attention_mask)
    position_ids = np.asarray(position_ids)
    Wq, Wk, Wv, Wo = (np.asarray(a) for a in (Wq, Wk, Wv, Wo))

    mode = detect_mode(attention_mask)
    nc = _get_nc(mode)

    in_maps = make_host_inputs(hidden_states, position_ids,
                               Wq, Wk, Wv, Wo, mode, S,
                               wo_res=BEST_OPTS.get("wo_res", False))
    if mode == "causal":
        cm = tri128()
        for m in in_maps:
            m["cmask"] = cm
    if mode == "general":
        mT = np.ascontiguousarray(
            attention_mask.reshape(S, S).T).astype(np.float32)
        for m in in_maps:
            m["maskT"] = mT

    res = run_bass_kernel_spmd(nc, in_maps, core_ids=list(range(NCORES)))

    acc = np.zeros((H, S), np.float32)
    for c in range(NCORES):
        acc += res.results[c]["outT"].astype(np.float32)
    return np.ascontiguousarray(acc.T).reshape(1, S, H).astype(np.float32)

